# revision 11
# baseline (speedup 1.0000x reference)
"""GPT-2 (L=12, B=4, T=1024, C=768, H=12, V=50257) forward pass on 8 TRN2
NeuronCores.

Sharding: 8 cores = 4 sequences x 2 "causal-balanced" halves.  Core (b, s)
owns query blocks [0,3,4,7] (s=0) or [1,2,5,6] (s=1) of 128 rows each, in
ascending order.  With this pairing, position p on either core needs only
the first p+1 key blocks from EACH core of the pair (prefix property), so
attention computes the causal ~62.5% of S/AV instead of the full T keys.
Per-layer pairwise AllGather of (K^T, V) within each sequence's core pair;
per-(rank, chunk) edge masks are per-core input data.

DMA strategy: weight loads coalesced (1 DMA per weight tensor per layer,
4 groups for w1/w2) and issued on the SP HWDGE queue; collective staging +
gather loads + output stores issued on the Activation HWDGE queue so the
two queues never head-of-line block each other.  All small per-layer
vectors are host-packed into two tensors loaded once.

Compute dtype: bf16 matmuls with fp32 PSUM accumulation; fp32 residual
stream, layernorm stats, and softmax accumulation.
"""

import numpy as np
import ml_dtypes

import concourse.bass as bass
import concourse.tile as tile
import concourse.mybir as mybir
from concourse import bacc, bass_utils
from concourse.masks import make_identity

F32 = mybir.dt.float32
BF16 = mybir.dt.bfloat16
AF = mybir.ActivationFunctionType
OP = mybir.AluOpType

L, B, T, C, H, V = 12, 4, 1024, 768, 12, 50257
D = C // H            # 64
FF = 4 * C            # 3072
R = 512               # rows per core
NC_ = 8               # cores
KO = C // 128         # 6
RT = R // 128         # 4 row tiles (= positions)
HP = H // 2           # 6 head pairs
NVS = 99              # vocab slices of 512
VP = NVS * 512        # 50688 padded vocab
SCALE = 1.0 / float(np.sqrt(D))
EPS = 1e-5

BLKS = ([0, 3, 4, 7], [1, 2, 5, 6])   # abs q-blocks per pair rank

# per-layer f32 small-vector pack layout (per partition):
#   [bq 6 | bk 6 | g1 6 | h1 6 | g2 6 | h2 6 | b1 24] = 60 per layer,
#   then [lnf_g 6 | lnf_b 6]
SV_L = 60
SV_TOT = L * SV_L + 12

# attention pt (exp'd scores) free-dim layout: per rank, chunks c=0..3 have
# query-suffix widths 512,384,256,128 at offsets below; rank 1 at +1280
PT_OFF = [0, 512, 896, 1152]
PT_RANK = 1280

_BUILD_CACHE = {}


def _build_nc(sim=False):
    nc = bacc.Bacc("TRN2", target_bir_lowering=False, debug=False,
                   num_devices=NC_)

    # ---- I/O ----
    x0_d = nc.dram_tensor("x0", [R, C], F32, kind="ExternalInput")
    mask_d = nc.dram_tensor("maskt", [128, 2, RT, 128], BF16,
                            kind="ExternalInput")
    wq_d = nc.dram_tensor("wq", [L, KO, 128, C], BF16, kind="ExternalInput")
    wk_d = nc.dram_tensor("wk", [L, KO, 128, C], BF16, kind="ExternalInput")
    wv_d = nc.dram_tensor("wv", [L, 128, KO, C], BF16, kind="ExternalInput")
    wo_d = nc.dram_tensor("wo", [L, 128, HP, C], BF16, kind="ExternalInput")
    w1_d = nc.dram_tensor("w1", [L, FF // 128, 128, C], BF16,
                          kind="ExternalInput")
    w2_d = nc.dram_tensor("w2", [L, FF, C], BF16, kind="ExternalInput")
    sv_d = nc.dram_tensor("smallv", [128, SV_TOT], F32, kind="ExternalInput")
    lb_d = nc.dram_tensor("lbias", [L, 3 * C], BF16, kind="ExternalInput")
    hw_d = nc.dram_tensor("headw", [128, KO, VP], BF16, kind="ExternalInput")
    out_d = nc.dram_tensor("logits", [R, V], F32, kind="ExternalOutput")

    KVLEN = C * R                 # kT elems
    VLEN = 128 * RT * H * 65      # v elems
    with tile.TileContext(nc) as tc:
        with (
            tc.tile_pool(name="const", bufs=1) as const,
            tc.tile_pool(name="persist", bufs=1) as persist,
            tc.tile_pool(name="wpool", bufs=1) as wpool,
            tc.tile_pool(name="mlpw", bufs=2) as mlpw,
            tc.tile_pool(name="attn", bufs=1) as attnp,
            tc.tile_pool(name="act1", bufs=2) as act1,
            tc.tile_pool(name="ps", bufs=2, space="PSUM") as ps,
            tc.tile_pool(name="dram", bufs=2, space="DRAM") as dramp,
        ):
            ident = const.tile([128, 128], BF16)
            make_identity(nc, ident[:])
            eps_t = const.tile([128, 1], F32)
            nc.vector.memset(eps_t[:], EPS)
            ones_r = const.tile([1, 128], BF16)
            nc.vector.memset(ones_r[:], 1.0)
            mask_sb = const.tile([128, 2, RT, 128], BF16)
            nc.sync.dma_start(mask_sb[:], mask_d.ap())
            sv = const.tile([128, SV_TOT], F32)
            nc.sync.dma_start(sv[:], sv_d.ap())

            # residual stream, fp32, [128, rt, C]
            x_sb = persist.tile([128, RT, C], F32)
            nc.sync.dma_start(
                x_sb[:], x0_d.ap().rearrange("(o p) c -> p o c", p=128))

            def layernorm_T(gT, bT, tag):
                """LN(x) -> bf16, transposed into hT [128, KO, R].

                gT/bT are [128, KO] partition-form gain/bias, applied after
                the transpose (feature dim lands on partitions there)."""
                hT = attnp.tile([128, KO, R], BF16, tag="ht")
                for rt in range(RT):
                    xs = x_sb[:, rt, :]
                    stats = act1.tile([128, 3, 6], F32, tag=f"st{tag}")
                    xs3 = xs.rearrange("p (s d) -> p s d", s=3)
                    for s in range(3):
                        nc.vector.bn_stats(stats[:, s, :], xs3[:, s, :])
                    mv = act1.tile([128, 2], F32, tag=f"mv{tag}")
                    nc.vector.bn_aggr(mv[:], stats[:])
                    std = act1.tile([128, 1], F32, tag=f"sd{tag}")
                    nc.scalar.activation(std[:], mv[:, 1:2], AF.Sqrt,
                                         bias=eps_t[:])
                    nc.vector.reciprocal(std[:], std[:])
                    hnb = act1.tile([128, C], BF16, tag=f"hb{tag}")
                    nc.vector.tensor_scalar(hnb[:], xs, mv[:, 0:1], std[:],
                                            op0=OP.subtract, op1=OP.mult)
                    for ko in range(KO):
                        pt_ = ps.tile([128, 512], BF16, tag="s")
                        nc.tensor.transpose(pt_[:, :128],
                                            hnb[:, ko * 128:(ko + 1) * 128],
                                            ident[:])
                        nc.vector.tensor_scalar(
                            hT[:, ko, rt * 128:(rt + 1) * 128], pt_[:, :128],
                            gT[:, ko:ko + 1], bT[:, ko:ko + 1],
                            op0=OP.mult, op1=OP.add)
                return hT

            for l in range(L):
                svb = l * SV_L
                bqT = sv[:, svb + 0:svb + 6]
                bkT = sv[:, svb + 6:svb + 12]
                g1T = sv[:, svb + 12:svb + 18]
                h1T = sv[:, svb + 18:svb + 24]
                g2T = sv[:, svb + 24:svb + 30]
                h2T = sv[:, svb + 30:svb + 36]
                b1T = sv[:, svb + 36:svb + 60]

                # --- coalesced per-layer weight loads (SP queue) ---
                lbias = act1.tile([1, 3 * C], BF16, tag="lb")
                nc.sync.dma_start(lbias[:], lb_d.ap()[l][None, :])
                wk_sb = wpool.tile([128, KO, KO, 128], BF16, tag="wk")
                nc.sync.dma_start(
                    wk_sb[:], wk_d.ap()[l].rearrange(
                        "m p (ko j) -> p m ko j", ko=KO))
                wv_sb = wpool.tile([128, KO, C], BF16, tag="wv", bufs=2)
                nc.sync.dma_start(wv_sb[:], wv_d.ap()[l])
                wq_sb = wpool.tile([128, KO, KO, 128], BF16, tag="wq")
                nc.sync.dma_start(
                    wq_sb[:], wq_d.ap()[l].rearrange(
                        "m p (ko j) -> p m ko j", ko=KO))
                wo_sb = wpool.tile([128, HP, C], BF16, tag="wo")
                nc.sync.dma_start(wo_sb[:], wo_d.ap()[l])

                # --- LN1 -> hT ---
                hT = layernorm_T(g1T, h1T, "a")

                # --- kT = (h Wk)^T + bk ---  [128, KO, R]
                kT = attnp.tile([128, KO, R], BF16, tag="kt")
                for m in range(KO):
                    acc = ps.tile([128, 512], F32, tag="s")
                    for k in range(KO):
                        nc.tensor.matmul(acc[:], wk_sb[:, m, k, :],
                                         hT[:, k, :],
                                         start=(k == 0), stop=(k == KO - 1))
                    nc.vector.tensor_scalar_add(kT[:, m, :], acc[:],
                                                bkT[:, m:m + 1])
                kv_in = dramp.tile([KVLEN + VLEN], BF16, tag="kin")
                nc.scalar.dma_start(
                    kv_in[:KVLEN].rearrange("(ko p r) -> p ko r",
                                            p=128, r=R),
                    kT[:])
                kv_out = dramp.tile([2, KVLEN + VLEN], BF16, tag="kout")

                # --- v = h Wv + bv ---  [128, RT, H, 65] with ones col
                v_sb = attnp.tile([128, RT, H, 65], BF16, tag="v")
                nc.vector.memset(v_sb[:, :, :, 64:65], 1.0)
                for pair in range(2):
                    bbank = ps.tile([128, 512], F32, tag="bb", bufs=2,
                                    name=f"vbb{pair}")
                    for half in range(2):
                        rt = 2 * pair + half
                        acc = ps.tile([128, 512], F32, tag="s")
                        accb = bbank[:, half * 256:half * 256 + 256]
                        for k in range(KO):
                            hs = hT[:, k, rt * 128:(rt + 1) * 128]
                            nc.tensor.matmul(acc[:], hs, wv_sb[:, k, 0:512],
                                             start=(k == 0), stop=False)
                            nc.tensor.matmul(accb[:], hs, wv_sb[:, k, 512:C],
                                             start=(k == 0 and half == 0),
                                             stop=False)
                        nc.tensor.matmul(acc[:], ones_r[:], lbias[:, 0:512],
                                         start=False, stop=True)
                        nc.tensor.matmul(accb[:], ones_r[:],
                                         lbias[:, 512:C],
                                         start=False, stop=(half == 1))
                        nc.vector.tensor_copy(
                            v_sb[:, rt, 0:8, 0:64],
                            acc[:].rearrange("p (h d) -> p h d", d=64))
                    nc.vector.tensor_copy(
                        v_sb[:, 2 * pair:2 * pair + 2, 8:12, 0:64],
                        bbank[:].rearrange("p (t h d) -> p t h d",
                                           t=2, d=64))
                nc.gpsimd.dma_start(
                    kv_in[KVLEN:].rearrange("(o p x) -> p o x",
                                            p=128, x=H * 65),
                    v_sb[:])
                if sim:
                    for rank in range(2):
                        nc.scalar.dma_start(kv_out[rank], kv_in[:])
                else:
                    nc.gpsimd.collective_compute(
                        "AllGather", OP.bypass,
                        replica_groups=[[0, 1], [2, 3], [4, 5], [6, 7]],
                        ins=[kv_in[:].opt()], outs=[kv_out[:].opt()])

                # --- qT = (h Wq)^T + bq ---
                qT = attnp.tile([128, KO, R], BF16, tag="qt")
                for m in range(KO):
                    acc = ps.tile([128, 512], F32, tag="s")
                    for k in range(KO):
                        nc.tensor.matmul(acc[:], wq_sb[:, m, k, :],
                                         hT[:, k, :],
                                         start=(k == 0), stop=(k == KO - 1))
                    nc.vector.tensor_scalar_add(qT[:, m, :], acc[:],
                                                bqT[:, m:m + 1])

                # --- gathered kT / v (rank-major; prefix-causal per rank) ---
                kTg = attnp.tile([128, KO, 2, R], BF16, tag="ktg")
                vext = attnp.tile([128, 2, RT, H, 65], BF16, tag="vext")
                for rank, eng in ((0, nc.scalar), (1, nc.sync)):
                    eng.dma_start(
                        kTg[:, :, rank, :],
                        kv_out[rank][:KVLEN]
                        .rearrange("(ko p r) -> p ko r", p=128, r=R))
                for rank, eng in ((0, nc.scalar), (1, nc.sync)):
                    eng.dma_start(
                        vext[:, rank, :, :, :],
                        kv_out[rank][KVLEN:]
                        .rearrange("(o p h e) -> p o h e", p=128, h=H, e=65))

                # --- attention ---
                # For query position p (0..3) and either rank, only key
                # chunks c <= p are causally live.  S over chunk (r, c) is
                # computed for the query suffix [128c : 512]; the first 128
                # columns of each chunk (its "edge" position p == c) get the
                # per-core mask.  AV accumulates per position.
                yT2 = attnp.tile([128, HP, R], BF16, tag="yt")
                for hp in range(HP):
                    yas = [ps.tile([128, 512], F32, tag="ya", bufs=4,
                                    name=f"ya{i}") for i in range(2)]
                    pts = [attnp.tile([128, 2 * PT_RANK], BF16, tag="pt",
                                      bufs=3, name=f"pt{i}")
                           for i in range(2)]
                    for r in range(2):
                        for c in range(RT):
                            qs = 128 * c
                            n = 512 - qs
                            off = r * PT_RANK + PT_OFF[c]
                            for sub in range(2):
                                p0 = 64 * sub
                                s = ps.tile([128, 512], F32, tag="s")
                                nc.tensor.matmul(
                                    s[:, :n],
                                    kTg[p0:p0 + 64, hp, r,
                                        c * 128:(c + 1) * 128],
                                    qT[p0:p0 + 64, hp, qs:512],
                                    start=True, stop=True)
                                nc.scalar.activation(
                                    pts[sub][:, off:off + n], s[:, :n],
                                    AF.Exp, scale=SCALE)
                                nc.vector.tensor_mul(
                                    pts[sub][:, off:off + 128],
                                    pts[sub][:, off:off + 128],
                                    mask_sb[:, r, c, :])
                    av = [(p, r, c) for p in range(RT)
                          for r in range(2) for c in range(p + 1)]
                    for i, (p, r, c) in enumerate(av):
                        pcol = slice(p * 128, (p + 1) * 128)
                        off = r * PT_RANK + PT_OFF[c] + (p - c) * 128
                        for sub in range(2):
                            h = 2 * hp + sub
                            nc.tensor.matmul(
                                yas[sub][0:65, pcol],
                                vext[:, r, c, h, :],
                                pts[sub][:, off:off + 128],
                                start=(i == 0),
                                stop=(i == len(av) - 1))
                    for sub in range(2):
                        ya = yas[sub]
                        rl = act1.tile([1, R], F32, tag="rl")
                        nc.vector.reciprocal(rl[:], ya[64:65, :])
                        rlb = act1.tile([64, R], F32, tag="rlb")
                        nc.gpsimd.partition_broadcast(rlb[:], rl[:])
                        nc.vector.tensor_tensor(
                            yT2[64 * sub:64 * sub + 64, hp, :],
                            ya[0:64, :], rlb[:], OP.mult)

                # --- proj: x += yT2^T Wo + bo ---  (K=128 per head pair)
                for pair in range(2):
                    bbank = ps.tile([128, 512], F32, tag="bb", bufs=2,
                                    name=f"pbb{pair}")
                    for half in range(2):
                        rt = 2 * pair + half
                        acc = ps.tile([128, 512], F32, tag="ya", bufs=4)
                        accb = bbank[:, half * 256:half * 256 + 256]
                        for hp in range(HP):
                            ys = yT2[:, hp, rt * 128:(rt + 1) * 128]
                            nc.tensor.matmul(acc[:], ys, wo_sb[:, hp, 0:512],
                                             start=(hp == 0), stop=False)
                            nc.tensor.matmul(accb[:], ys,
                                             wo_sb[:, hp, 512:C],
                                             start=(hp == 0 and half == 0),
                                             stop=False)
                        nc.tensor.matmul(acc[:], ones_r[:],
                                         lbias[:, C:C + 512],
                                         start=False, stop=True)
                        nc.tensor.matmul(accb[:], ones_r[:],
                                         lbias[:, C + 512:2 * C],
                                         start=False, stop=(half == 1))
                        nc.vector.tensor_tensor(x_sb[:, rt, 0:512],
                                                x_sb[:, rt, 0:512],
                                                acc[:], OP.add)
                    nc.vector.tensor_tensor(
                        x_sb[:, 2 * pair:2 * pair + 2, 512:C],
                        x_sb[:, 2 * pair:2 * pair + 2, 512:C],
                        bbank[:].rearrange("p (t n) -> p t n", t=2), OP.add)

                # --- LN2 -> hT2 ---
                hT2 = layernorm_T(g2T, h2T, "b")

                # --- MLP: single streaming pass over FF tiles ---
                fca = [ps.tile([128, 512], F32, tag="ya", bufs=4,
                                name=f"fca{i}") for i in range(RT)]
                fbank = [ps.tile([128, 512], F32, tag="bb", bufs=2,
                                 name=f"fbb{i}") for i in range(2)]
                fcb = [fbank[i // 2][:, (i % 2) * 256:(i % 2) * 256 + 256]
                       for i in range(RT)]
                NM = FF // 128        # 24
                for m in range(NM):
                    if m % 4 == 0:
                        g = m // 4
                        w1g = mlpw.tile([128, 4, KO, 128], BF16, tag="w1")
                        nc.sync.dma_start(
                            w1g[:], w1_d.ap()[l, g * 4:(g + 1) * 4]
                            .rearrange("m p (ko j) -> p m ko j", ko=KO))
                        w2g = mlpw.tile([128, 4, C], BF16, tag="w2")
                        nc.sync.dma_start(
                            w2g[:], w2_d.ap()[l, g * 512:(g + 1) * 512]
                            .rearrange("(m p) c -> p m c", p=128))
                    mi = m % 4
                    gacc = ps.tile([128, 512], F32, tag="s")
                    for k in range(KO):
                        nc.tensor.matmul(gacc[:], w1g[:, mi, k, :],
                                         hT2[:, k, :],
                                         start=(k == 0), stop=(k == KO - 1))
                    gt_ = act1.tile([128, 512], BF16, tag="g", bufs=3)
                    nc.scalar.activation(gt_[:], gacc[:], AF.Gelu,
                                         bias=b1T[:, m:m + 1])
                    for rt in range(RT):
                        gs = gt_[:, rt * 128:(rt + 1) * 128]
                        nc.tensor.matmul(fca[rt][:], gs, w2g[:, mi, 0:512],
                                         start=(m == 0), stop=False)
                        nc.tensor.matmul(fcb[rt][:], gs, w2g[:, mi, 512:C],
                                         start=(m == 0 and rt % 2 == 0),
                                         stop=False)
                for rt in range(RT):
                    nc.tensor.matmul(fca[rt][:], ones_r[:],
                                     lbias[:, 2 * C:2 * C + 512],
                                     start=False, stop=True)
                    nc.tensor.matmul(fcb[rt][:], ones_r[:],
                                     lbias[:, 2 * C + 512:3 * C],
                                     start=False, stop=(rt % 2 == 1))
                    nc.vector.tensor_tensor(x_sb[:, rt, 0:512],
                                            x_sb[:, rt, 0:512],
                                            fca[rt][:], OP.add)
                for pair in range(2):
                    nc.vector.tensor_tensor(
                        x_sb[:, 2 * pair:2 * pair + 2, 512:C],
                        x_sb[:, 2 * pair:2 * pair + 2, 512:C],
                        fbank[pair][:].rearrange("p (t n) -> p t n", t=2),
                        OP.add)

            # ---- final LN + head ----
            gfT = sv[:, L * SV_L:L * SV_L + 6]
            hfT = sv[:, L * SV_L + 6:L * SV_L + 12]
            xfT = layernorm_T(gfT, hfT, "a")
            out_r = out_d.ap().rearrange("(o p) v -> p o v", p=128)
            for vs in range(NVS):
                if vs % 2 == 0:
                    hwv = wpool.tile([128, KO, 1024], BF16, tag="wv",
                                     bufs=2)
                    vw2 = min(1024, VP - vs * 512)
                    nc.sync.dma_start(
                        hwv[:, :, :vw2],
                        hw_d.ap()[:, :, vs * 512:vs * 512 + vw2])
                vi = (vs % 2) * 512
                st = attnp.tile([128, RT, 512], F32, tag="pt", bufs=3)
                for rt in range(RT):
                    acc = ps.tile([128, 512], F32, tag="ya", bufs=4)
                    for k in range(KO):
                        nc.tensor.matmul(acc[:],
                                         xfT[:, k, rt * 128:(rt + 1) * 128],
                                         hwv[:, k, vi:vi + 512],
                                         start=(k == 0), stop=(k == KO - 1))
                    if rt < 2:
                        nc.vector.tensor_copy(st[:, rt, :], acc[:])
                    else:
                        nc.scalar.copy(st[:, rt, :], acc[:])
                vw = min(512, V - vs * 512)
                nc.scalar.dma_start(
                    out_r[:, :, vs * 512:vs * 512 + vw], st[:, :, :vw])

    nc.compile()
    return nc


def _prep_inputs(inputs):
    f = lambda k: np.asarray(inputs[k], dtype=np.float32)
    bf = lambda k: np.ascontiguousarray(
        np.asarray(inputs[k], dtype=np.float32)).astype(ml_dtypes.bfloat16)

    idx = np.asarray(inputs["idx"])
    tok = f("tok_emb")
    pos = f("pos_emb")[0]                      # [T, C]
    x0 = tok[idx] + pos[None, :, :]            # [B, T, C] f32

    hw = np.zeros((128, KO, VP), dtype=ml_dtypes.bfloat16)
    hw[:, :, :V] = bf("head_w").reshape(KO, 128, V).transpose(1, 0, 2)

    def pack_kT(w):            # [L, C, C] -> [L, KO(m), 128(p), (ko j)]
        a = w.reshape(L, KO, 128, KO, 128)         # (l, ko, p, m, j)
        return np.ascontiguousarray(a.transpose(0, 3, 2, 1, 4)).reshape(
            L, KO, 128, C)

    def pack_w1(w):            # [L, C, FF] -> [L, FF//128(m), 128(p), (ko j)]
        a = w.reshape(L, KO, 128, FF // 128, 128)  # (l, ko, p, m, j)
        return np.ascontiguousarray(a.transpose(0, 3, 2, 1, 4)).reshape(
            L, FF // 128, 128, C)

    def pack_rhs(w):           # [L, K, N] -> [L, 128(p), K//128(ko), N]
        a = w.reshape(L, KO, 128, C)
        return np.ascontiguousarray(a.transpose(0, 2, 1, 3))

    def pack_wo(w):            # [L, C, C] -> [L, 128, HP, C] head-pair packed
        a = w.reshape(L, HP, 2, 64, C)             # (l, hp, sub, d, c)
        return np.ascontiguousarray(a.transpose(0, 2, 3, 1, 4)).reshape(
            L, 128, HP, C)

    def colpack(v):            # [L, X*128] f32 -> [128, L, X] partition-form
        X = v.shape[1] // 128
        return v.reshape(L, X, 128).transpose(2, 0, 1)

    sv = np.zeros((128, SV_TOT), dtype=np.float32)
    svv = sv[:, :L * SV_L].reshape(128, L, SV_L)
    svv[:, :, 0:6] = colpack(f("bq"))
    svv[:, :, 6:12] = colpack(f("bk"))
    svv[:, :, 12:18] = colpack(f("ln1_g"))
    svv[:, :, 18:24] = colpack(f("ln1_b"))
    svv[:, :, 24:30] = colpack(f("ln2_g"))
    svv[:, :, 30:36] = colpack(f("ln2_b"))
    svv[:, :, 36:60] = colpack(f("b1"))
    sv[:, L * SV_L:L * SV_L + 6] = f("lnf_g").reshape(6, 128).T
    sv[:, L * SV_L + 6:L * SV_L + 12] = f("lnf_b").reshape(6, 128).T

    lb = np.concatenate([bf("bv"), bf("bo"), bf("b2")], axis=1)  # [L, 3C]

    shared = {
        "wq": pack_kT(bf("wq")), "wk": pack_kT(bf("wk")),
        "wv": pack_rhs(bf("wv")), "wo": pack_wo(bf("wo")),
        "w1": pack_w1(bf("w1")), "w2": bf("w2"),
        "smallv": sv, "lbias": np.ascontiguousarray(lb),
        "headw": hw,
    }

    in_maps = []
    for core in range(NC_):
        b, s = core // 2, core % 2
        own, other = BLKS[s], BLKS[1 - s]
        rows = np.concatenate(
            [np.arange(blk * 128, (blk + 1) * 128) for blk in own])
        # maskt[r][p]: edge mask for key chunk p of rank r vs own q block p
        m = np.zeros((2, RT, 128, 128), dtype=ml_dtypes.bfloat16)
        dd = np.arange(128)
        for p in range(RT):
            m[s, p] = (dd[:, None] <= dd[None, :])        # own rank: diag
            m[1 - s, p] = float(other[p] < own[p])        # other: all or none
        in_maps.append(dict(
            shared,
            x0=np.ascontiguousarray(x0[b, rows]),
            maskt=np.ascontiguousarray(m.transpose(2, 0, 1, 3)),
        ))
    return in_maps


def kernel(**inputs):
    if "nc" not in _BUILD_CACHE:
        _BUILD_CACHE["nc"] = _build_nc()
    nc = _BUILD_CACHE["nc"]

    in_maps = _prep_inputs(inputs)
    res = bass_utils.run_bass_kernel_spmd(
        nc, in_maps, core_ids=list(range(NC_)))

    out = np.empty((B, T, V), dtype=np.float32)
    for core in range(NC_):
        b, s = core // 2, core % 2
        rows = np.concatenate(
            [np.arange(blk * 128, (blk + 1) * 128) for blk in BLKS[s]])
        out[b, rows] = res.results[core]["logits"]
    return out
